# revision 29
# baseline (speedup 1.0000x reference)
"""MLA (absorbed-weight multi-head latent attention) TRN2 Bass kernel, v3.

Problem: B=2, N=NKV=2048, E=4096, H=16, HD=256, LQ=512, LKV=256.
  C_q  = Q @ Wq_d                 [B,N,LQ]
  C_kv = K @ Wkv_d                [B,Nkv,LKV]
  CqWqk = (C_q @ W_qk)            [B,N,H,LKV]
  scores = einsum('bnhl,bkl->bhnk', CqWqk, C_kv) / sqrt(LKV)
  attn = softmax(scores, -1)
  out  = einsum('bhnk,bkl,lhd->bnhd', attn, C_kv, Wv_u)  (absorbed V)

Sharding: 8 cores = (batch b in 0..1) x (query quarter q in 0..3).
Each core handles n-rows [q*512,(q+1)*512) of batch b for ALL heads.

v3 design:
- All inputs bf16 (host-converted): ~32MB HBM traffic/core, FWL weight loads.
  PSUM accumulation fp32; numpy-validated fro rel err ~4.5e-3 (gate 2e-2).
- Projections stream in 4-E-chunk DMA batches; input DMAs demand-ordered on
  the SP HWDGE queue: QT/WQD -> WQK -> WKVD -> KT -> WVU.
- CqWqk for ALL heads computed between the two projection phases: turns the
  DMA-bound projection window PE-bound and shortens the per-head loop.
- Output side absorbed: O = P @ [C_kv | 1 1] accumulates the PV product in
  latent space AND the softmax denominator (col 256); after normalization O
  is DMA-transposed (XBAR, on the qAct queue) and projected by Wv_u_h.
Device dataflow (per core, T = transposed):
  C_qT   [LQ, n]   = lhsT Wq_d   @ rhs Q^T      (contract E, streamed)
  CqWqkT [LKV,H,n] = lhsT W_qk_h @ rhs C_qT     (contract LQ, all heads)
  C_kvT  [LKV, k]  = lhsT Wkv_d  @ rhs K^T      (contract E, streamed)
  ckv_aug[k, 258]  = DMA-transpose(C_kvT) | ones
  S^T    [k, n]    = lhsT C_kvT  @ rhs CqWqkT_h (contract LKV)
  P^T    = exp(S^T / 16)  (no max-subtraction: |S| <= ~6, fp32-safe)
  O      [n, 258]  = lhsT P^T    @ rhs ckv_aug  (contract k; col 256 = den)
  Obar   = O[:, :256] * recip(O[:, 256])        (softmax normalize)
  ObarT  [LKV, n]  = DMA-transpose(Obar)
  out    [n, HD]   = lhsT ObarT  @ rhs Wv_u_h   (contract LKV)
"""
import numpy as np

B, N, NKV, E, H = 2, 2048, 2048, 4096, 16
HD, LQ, LKV = 256, 512, 256
NCORES = 8
NQ = N // 4          # 512 query rows per core
ECH = E // 128       # 32 e-chunks
KCH = NKV // 128     # 16 k-chunks
NCK = NQ // 128      # 4 n-chunks per core

_cache = {}


def build_nc(iters=1, stop_after="full", fake_ckv=False):
    import concourse.bass as bass
    from concourse import bacc
    import concourse.mybir as mybir
    import concourse.tile as tile

    dt = mybir.dt
    bf16 = dt.bfloat16
    f32 = dt.float32
    do_heads = stop_after == "full"

    nc = bacc.Bacc(None, target_bir_lowering=False)
    QT = nc.dram_tensor("QT", [E, NQ], bf16, kind="ExternalInput")
    KT = nc.dram_tensor("KT", [E, NKV], bf16, kind="ExternalInput")
    WQD = nc.dram_tensor("WQD", [E, LQ], bf16, kind="ExternalInput")
    WQK = nc.dram_tensor("WQK", [LQ, H * LKV], bf16, kind="ExternalInput")
    WKVD = nc.dram_tensor("WKVD", [E, LKV], bf16, kind="ExternalInput")
    WVU = nc.dram_tensor("WVU", [LKV, H * HD], bf16, kind="ExternalInput")
    OUT = nc.dram_tensor("OUT", [NQ, E], f32, kind="ExternalOutput")
    OUT_R = OUT.rearrange("(c p) e -> p c e", p=128)
    QT_R = QT.rearrange("(c p) n -> p c n", p=128)
    WQD_R = WQD.rearrange("(c p) l -> p c l", p=128)
    KT_R = KT.rearrange("(c p) n -> p c n", p=128)

    Exp = mybir.ActivationFunctionType.Exp

    with tile.TileContext(nc) as tc:
        with tc.tile_pool(name="persist", bufs=1) as persist:
            loop_ctx = tc.For_i(0, iters, 1,
                                hint_engines=(mybir.EngineType.PE,)) \
                if iters > 1 else None
            if loop_ctx is not None:
                loop_ctx.__enter__()

            ckvt = persist.tile([128, 2, NKV], bf16, tag="ckvt")    # C_kvT
            ckva = persist.tile([128, KCH, 258], bf16, tag="ckva")  # ckv|1|1
            cqwqk = persist.tile([128, 2, H, NQ], bf16, tag="cqwqk")
            wvu = persist.tile([128, 2, H * HD], bf16, tag="wvu")
            nc.vector.memset(ckva[:, :, 256:258], 1.0)

            with tc.tile_pool(name="ph12", bufs=1) as ph, \
                 tc.tile_pool(name="accp", bufs=1, space="PSUM") as accp, \
                 tc.tile_pool(name="qs", bufs=3) as qs, \
                 tc.tile_pool(name="ks", bufs=3) as ks:
                accs = [accp.tile([128, 512], f32, tag=f"a{i}", name=f"acc_{i}")
                        for i in range(8)]
                cqt = ph.tile([128, 4, NQ], bf16, tag="cqt")
                wqk = ph.tile([128, 4, H * LKV], bf16, tag="wqk")
                wkvd = ph.tile([128, ECH, LKV], bf16, tag="wkvd")

                # ---- phase 1: C_qT, streamed E-chunk batches (small first
                # batch for a fast pipeline start) ----
                batches = [(0, 1), (1, 3)] + [(4 * b, 4) for b in range(1, 8)]
                for ec0, nec in batches:
                    qt_c = qs.tile([128, 4, NQ], bf16, tag="qt")
                    nc.sync.dma_start(out=qt_c[:, 0:nec, :],
                                      in_=QT_R[:, ec0:ec0 + nec, :])
                    wqd_c = qs.tile([128, 4, LQ], bf16, tag="wqd")
                    nc.sync.dma_start(out=wqd_c[:, 0:nec, :],
                                      in_=WQD_R[:, ec0:ec0 + nec, :])
                    for e4 in range(nec):
                        ec = ec0 + e4
                        for lc in range(4):
                            nc.tensor.matmul(accs[lc],
                                             wqd_c[:, e4, lc * 128:(lc + 1) * 128],
                                             qt_c[:, e4, :],
                                             start=(ec == 0), stop=(ec == ECH - 1))
                for lc in range(4):
                    if lc % 2 == 0:
                        nc.vector.tensor_copy(cqt[:, lc, :], accs[lc])
                    else:
                        nc.scalar.copy(cqt[:, lc, :], accs[lc])

                # demand-ordered weight loads on the same SP queue
                nc.sync.dma_start(
                    out=wqk[:, :, 0:H * LKV // 2],
                    in_=WQK.rearrange("(c p) o -> p c o", p=128)[:, :, 0:H * LKV // 2])
                nc.sync.dma_start(
                    out=wqk[:, :, H * LKV // 2:],
                    in_=WQK.rearrange("(c p) o -> p c o", p=128)[:, :, H * LKV // 2:])
                nc.sync.dma_start(
                    out=wkvd, in_=WKVD.rearrange("(c p) l -> p c l", p=128))

                # ---- interphase: CqWqkT for ALL heads (PE-fills DMA window) ----
                for h in range(H):
                    for lkc in range(2):
                        ps = accs[4 + (h * 2 + lkc) % 4]
                        for lc in range(4):
                            nc.tensor.matmul(
                                ps,
                                wqk[:, lc, h * LKV + lkc * 128:
                                    h * LKV + (lkc + 1) * 128],
                                cqt[:, lc, :], start=(lc == 0), stop=(lc == 3))
                        nc.vector.tensor_copy(cqwqk[:, lkc, h, :], ps)

                # ---- phase 2: C_kvT, streamed in 4-E-chunk batches ----
                for eb in range(ECH // 4):
                    kt_c = ks.tile([128, 4, NKV], bf16, tag="kt")
                    nc.sync.dma_start(out=kt_c, in_=KT_R[:, eb * 4:(eb + 1) * 4, :])
                    for e4 in range(4):
                        ec = eb * 4 + e4
                        for lkc in range(2):
                            for nt in range(4):
                                nc.tensor.matmul(
                                    accs[lkc * 4 + nt],
                                    wkvd[:, ec, lkc * 128:(lkc + 1) * 128],
                                    kt_c[:, e4, nt * 512:(nt + 1) * 512],
                                    start=(ec == 0), stop=(ec == ECH - 1))
                nc.sync.dma_start(
                    out=wvu, in_=WVU.rearrange("(c p) o -> p c o", p=128))
                for nt in range(4):
                    for lkc in range(2):
                        dst = ckvt[:, lkc, nt * 512:(nt + 1) * 512]
                        if lkc == 0:
                            nc.vector.tensor_copy(dst, accs[lkc * 4 + nt])
                        else:
                            nc.scalar.copy(dst, accs[lkc * 4 + nt])

            # ckv_aug = transpose(ckvt) via DMA XBAR. The XBAR needs a
            # contiguous, aligned destination (strided dest is silently wrong
            # on HW), so transpose into contiguous staging then strided-copy
            # on DVE/ACT. Split per latent half and in 4+12 k-chunk pieces so
            # head 0's first PV chunks are ready promptly.
            ckv_t = persist.tile([128, 2, KCH, 128], bf16, tag="ckv_t")
            pieces = [(0, 4), (4, KCH)]
            for lkc in range(2):
                eng = nc.sync if lkc == 0 else nc.scalar
                for p0, p1 in pieces:
                    eng.dma_start_transpose(
                        out=ckv_t[:, lkc, p0:p1, :],
                        in_=ckvt[:, lkc, p0 * 128:p1 * 128])
            for lkc in range(2):
                eng = nc.vector if lkc == 0 else nc.scalar
                cp = eng.tensor_copy if lkc == 0 else eng.copy
                for p0, p1 in pieces:
                    cp(ckva[:, p0:p1, lkc * 128:(lkc + 1) * 128],
                       ckv_t[:, lkc, p0:p1, :])

            # ---------- phase 3: per-head attention ----------
            with tc.tile_pool(name="head", bufs=3) as hp, \
                 tc.tile_pool(name="ptp", bufs=4) as ptp, \
                 tc.tile_pool(name="psO", bufs=1, space="PSUM") as psO, \
                 tc.tile_pool(name="psS", bufs=3, space="PSUM") as psS, \
                 tc.tile_pool(name="psP", bufs=1, space="PSUM") as psP:
                for h in range(H):
                    if not do_heads:
                        dummy = ptp.tile([128, NCK, HD], f32, tag="ot")
                        nc.vector.memset(dummy, 0.5)
                        nc.sync.dma_start(
                            out=OUT_R[:, :, h * HD:(h + 1) * HD], in_=dummy)
                        continue
                    # scores^T -> exp -> O accumulate (with denominator col)
                    pso = [psO.tile([128, 258], f32, tag=f"o{i}",
                                    name=f"pso{i}") for i in range(NCK)]
                    for kc in range(KCH):
                        pss = psS.tile([128, 512], f32, tag="sw")
                        for lkc in range(2):
                            nc.tensor.matmul(
                                pss, ckvt[:, lkc, kc * 128:(kc + 1) * 128],
                                cqwqk[:, lkc, h, :],
                                start=(lkc == 0), stop=(lkc == 1))
                        pt = ptp.tile([128, NQ], bf16, tag="pt")
                        nc.scalar.activation(out=pt, in_=pss, func=Exp,
                                             scale=1.0 / 16.0)
                        for nk in range(NCK):
                            nc.tensor.matmul(
                                pso[nk], pt[:, nk * 128:(nk + 1) * 128],
                                ckva[:, kc, :],
                                start=(kc == 0), stop=(kc == KCH - 1))

                    # normalize -> Obar (bf16) -> DMA-transpose -> ObarT,
                    # in two nk-pair halves to shorten the per-head tail
                    obar = hp.tile([128, NCK, 256], bf16, tag="obar")
                    obarT = hp.tile([128, 8, 128], bf16, tag="obarT")
                    ot = ptp.tile([128, NCK, HD], f32, tag="ot")
                    po2 = psP.tile([128, 2, HD], f32, tag="po")
                    for half in range(2):
                        for nk in (2 * half, 2 * half + 1):
                            den = hp.tile([128, 1], f32, tag="den")
                            nc.vector.reciprocal(den, pso[nk][:, 256:257])
                            nc.vector.tensor_scalar_mul(obar[:, nk, :],
                                                        pso[nk][:, 0:256], den)
                        eng = nc.scalar if half == 0 else nc.sync
                        eng.dma_start_transpose(
                            out=obarT[:, 4 * half:4 * half + 4, :],
                            in_=obar[:, 2 * half:2 * half + 2, :])
                        for nk in (2 * half, 2 * half + 1):
                            po = po2[:, nk % 2, :]
                            for h2 in range(2):
                                nc.tensor.matmul(
                                    po, obarT[:, nk * 2 + h2, :],
                                    wvu[:, h2, h * HD:(h + 1) * HD],
                                    start=(h2 == 0), stop=(h2 == 1))
                            nc.vector.tensor_copy(ot[:, nk, :], po)
                    nc.sync.dma_start(
                        out=OUT_R[:, :, h * HD:(h + 1) * HD], in_=ot)

            if loop_ctx is not None:
                loop_ctx.__exit__(None, None, None)

    nc.finalize()
    return nc


def get_nc(iters=1, stop_after="full", fake_ckv=False):
    key = (iters, stop_after, fake_ckv)
    if key not in _cache:
        _cache[key] = build_nc(iters, stop_after, fake_ckv)
    return _cache[key]


def make_in_maps(Q, K, Wq_d, W_qk, Wkv_d, Wv_u):
    import ml_dtypes
    bf = ml_dtypes.bfloat16
    Q = np.asarray(Q, dtype=np.float32)
    K = np.asarray(K, dtype=np.float32)
    weights = {
        "WQD": np.ascontiguousarray(np.asarray(Wq_d, dtype=bf)),
        "WQK": np.ascontiguousarray(np.asarray(W_qk, dtype=bf)),
        "WKVD": np.ascontiguousarray(np.asarray(Wkv_d, dtype=bf)),
        "WVU": np.ascontiguousarray(np.asarray(Wv_u, dtype=bf)),
    }
    kts = [np.ascontiguousarray(K[b].T.astype(bf)) for b in range(B)]
    qts = [np.ascontiguousarray(Q[b].T.astype(bf)) for b in range(B)]
    in_maps = []
    for c in range(NCORES):
        b, q = divmod(c, 4)
        m = dict(weights)
        m["KT"] = kts[b]
        m["QT"] = np.ascontiguousarray(qts[b][:, q * NQ:(q + 1) * NQ])
        in_maps.append(m)
    return in_maps


def kernel(Q, K, Wq_d, W_qk, Wkv_d, Wv_u):
    from concourse.bass_utils import run_bass_kernel_spmd

    nc = get_nc(1)
    in_maps = make_in_maps(Q, K, Wq_d, W_qk, Wkv_d, Wv_u)
    res = run_bass_kernel_spmd(nc, in_maps, core_ids=list(range(NCORES)))
    out = np.empty((B, N, E), dtype=np.float32)
    for c in range(NCORES):
        b, q = divmod(c, 4)
        out[b, q * NQ:(q + 1) * NQ, :] = res.results[c]["OUT"]
    return out
